# revision 36
# baseline (speedup 1.0000x reference)
"""DynamicLinear (MoE routing) Trainium2 Bass kernel.

Math (per sample b):
    out[b] = sum_k attn[b,k] * (x[b] @ W[k].T + bias[k])
           = sum_k attn[b,k] * (x[b] @ W[k].T) + attn[b] @ bias

Sharding: 8 cores in a 2x4 grid over (batch, out_features).
Each core computes out[b_half, o_quarter] from x[b_half] (8 MiB bf16)
and W[:, o_quarter, :] (8 MiB bf16) -- no cross-core communication.

The host ships x and W pre-tiled and pre-cast to bf16 in the exact
SBUF layouts the kernel consumes (contraction dim on partitions,
contiguous per partition), and attn pre-tiled / bias pre-replicated,
so the device needs no casts, transposes, or gathers. Matmuls run
bf16 x bf16 with fp32 PSUM accumulation (the compute roofline:
218.6 us/core; fp8 DoubleRow is only ~1.44x on TRN2 and needs >=3
matmuls to stay under the 2e-2 error budget, a net loss).

Schedule (242 us measured vs 248-251 us baseline; PE-busy is within
~1 us of the warm roofline, the rest is fixed preamble ~8 us, the
DMA-bandwidth-bound fill ~4 us, and drain tail ~4.5 us):
- ~4.3 us of zero-matmul warmup + small dummy groups woven into the
  fill keep the PE busy through a full 3.4-us HAM window, so the
  clock gate flips to 8/8 (2.4 GHz) before the dense phase.
- Expert-0 weights stream in 8 x 2-ii granules; the first three k=0
  passes are skewed over the granule stream (lags 0/1/3) so each
  arriving granule feeds 6 matmuls and the PE does 3 passes of real
  work during the fill instead of stalling on W0.
- Queue plan (engines round-robin across the two HWDGE queues, FIFO
  within each): scalar carries only the bytes needed early (x0, x1,
  attn, x2, x3), then W1 and bias which land before their ~68 us
  consumers; sync carries W0 at full remaining rate, then x4..x15
  riding ahead of k=0 consumption, then W2/W3, then the k=3 stores.
  Nothing rides the slow SWDGE (gpsimd) path, and bias is shipped
  host-pre-replicated (1 MiB plain load) instead of an on-device
  [0,128] broadcast (~52 us of shared engine time).
- The bias madds are deferred to the k=1/k=2 combines: off both the
  fill window and the final-store tail.
- The last-processed tile (k=3, t=15) runs as 384/128-column half
  passes with separate PSUM tiles, accs, and store queues, so the
  tail after the last matmul is one 128-column combine + 64 KiB
  store before the fixed drain.
"""

import numpy as np

_B, _K, _IN, _OUT = 4096, 4, 2048, 2048
_GRID_B, _GRID_O = 2, 4
_BL = _B // _GRID_B      # 2048 batch rows per core
_OL = _OUT // _GRID_O    # 512 out cols per core
_NBT = _BL // 128        # 16 b tiles
_NIT = _IN // 128        # 16 contraction tiles

_CACHE = {}
LAST_RESULTS = None


def _build_program():
    import concourse.bass as bass
    import concourse.tile as tile
    from concourse import bacc, mybir

    f32 = mybir.dt.float32
    MULT = mybir.AluOpType.mult
    ADD = mybir.AluOpType.add

    nc = bacc.Bacc("TRN2", target_bir_lowering=False, debug=False)
    bf16 = mybir.dt.bfloat16
    xT = nc.dram_tensor("xT", [_NBT, 128, _NIT, 128], bf16,
                        kind="ExternalInput").ap()
    # host-pretiled: attn[p, t, k] = softmax_attention[t*128 + p, k];
    # a plain contiguous load (the [b, k] layout needs a 2048-descriptor
    # gather whose DIRECT2D generation alone costs 10-18 us)
    attn = nc.dram_tensor("attn", [128, _NBT, _K], f32,
                          kind="ExternalInput").ap()
    wT = nc.dram_tensor("wT", [_K, 128, _NIT, _OL], bf16,
                        kind="ExternalInput").ap()
    # host-pre-replicated across partitions: a plain 1 MiB HWDGE load
    # (an on-device [0,128] broadcast runs on the slow SWDGE path and
    # costs ~52 us of shared DMA-engine time right in the fill window)
    bias = nc.dram_tensor("bias", [128, _K, _OL], f32,
                          kind="ExternalInput").ap()
    out = nc.dram_tensor("out", [_BL, _OL], f32, kind="ExternalOutput").ap()

    G0 = 4            # ii-tiles per W granule, expert 0 (4 KiB/partition
                      # descriptors -- parity with x tiles in the DMA
                      # engines' per-descriptor round-robin)
    GH = 4            # ii-tiles per W granule, experts 1..3
    _SPLIT = 384      # column split of the final (k=3, t=15) pass

    with tile.TileContext(nc) as tc:
        with (
            tc.tile_pool(name="wt0", bufs=_NIT // G0) as wt0p,
            tc.tile_pool(name="wt", bufs=3 * (_NIT // GH)) as wtp,
            tc.tile_pool(name="xt", bufs=_NBT) as xtp,
            tc.tile_pool(name="singles", bufs=1) as singles,
            tc.tile_pool(name="acc", bufs=_NBT - 1) as accp,
            tc.tile_pool(name="acc15", bufs=1) as acc15p,
            tc.tile_pool(name="psum", bufs=6, space="PSUM") as psump,
            tc.tile_pool(name="psumh", bufs=1, space="PSUM") as psumhp,
        ):
            # --- PE warmup: ~3.4 us of zero matmuls bridging the PE from
            # sequencer-ready (~8 us) to first-granule-ready (~11 us);
            # together with the skewed fill passes below the PE then
            # stays busy through a full 3.4-us HAM window, flipping the
            # clock gate to 8/8 early.
            _NWARM = 20
            warm = singles.tile([128, 512], bf16, name="warm")
            nc.vector.memset(warm, 0.0)
            ps_warm = psump.tile([128, 256], f32, tag="ps", name="ps_warm")
            for i in range(_NWARM):
                nc.tensor.matmul(
                    ps_warm, lhsT=warm[:, 0:128], rhs=warm[:, 0:256],
                    start=(i == 0), stop=(i == _NWARM - 1),
                )

            # --- loads ---
            def load_w0(h):
                # expert-0 granule: wt0[h][i_in, j, o], j in [0, G0)
                t_ = wt0p.tile([128, G0, _OL], bf16, tag="wt0",
                               name=f"wt0_{h}")
                nc.sync.dma_start(out=t_, in_=wT[0, :, h * G0:(h + 1) * G0])
                return t_

            def load_w(k, h, q=None):
                t_ = wtp.tile([128, GH, _OL], bf16, tag="wt",
                              name=f"wt{k}_{h}")
                (q or nc.sync).dma_start(
                    out=t_, in_=wT[k, :, h * GH:(h + 1) * GH])
                return t_

            def load_x(t, q=None):
                # xt[t][i_in, ii, b] = x[t*128 + b, ii*128 + i_in]
                t_ = xtp.tile([128, _NIT, 128], bf16, tag="xt",
                              name=f"xt{t}")
                (q or nc.scalar).dma_start(out=t_, in_=xT[t])
                return t_

            # Queue plan (engines round-robin between the two HWDGE
            # queues, so each queue's FIFO order IS its data order):
            #   scalar: x0, x1, attn -- the only bytes needed in the
            #           first ~16 us -- then W1 and bias, which land
            #           well before their ~68 us consumers.
            #   sync:   W0 at full remaining rate (the fill-phase
            #           critical path), then the rest of the x stream
            #           riding ahead of k=0 consumption, then W2/W3,
            #           then the k=3 output stores.
            wt0 = {h: load_w0(h) for h in range(_NIT // G0)}
            xts = {0: load_x(0), 1: load_x(1)}
            attn_sb = singles.tile([128, _NBT, _K], f32)
            nc.scalar.dma_start(out=attn_sb, in_=attn)
            xts[2] = load_x(2)
            xts[3] = load_x(3)
            wt = {}
            for h in range(_NIT // GH):
                wt[(1, h)] = load_w(1, h, nc.scalar)
            bias_rep = singles.tile([128, _K, _OL], f32)
            nc.scalar.dma_start(out=bias_rep, in_=bias)
            for t in range(4, _NBT):
                xts[t] = load_x(t, nc.sync)
            for k in (2, 3):
                for h in range(_NIT // GH):
                    wt[(k, h)] = load_w(k, h)

            def w_slice(k, ii, c0=0, c1=_OL):
                if k == 0:
                    return wt0[ii // G0][:, ii % G0, c0:c1]
                return wt[(k, ii // GH)][:, ii % GH, c0:c1]

            acc = [None] * _NBT      # full tiles for t < 15
            acc15 = [None, None]     # [0:_SPLIT], [_SPLIT:_OL] for t = 15

            def combine(k, t, ps_ap, a_sc, c0, c1, which):
                # acc update for columns [c0:c1); which selects the acc
                at = acc[t] if t < _NBT - 1 else acc15[which]
                if k == 0:
                    # init: acc = a_0 * psum  (bias terms deferred)
                    nc.vector.tensor_scalar(
                        out=at, in0=ps_ap, scalar1=a_sc[:, 0:1],
                        scalar2=None, op0=MULT,
                    )
                else:
                    nc.vector.scalar_tensor_tensor(
                        out=at, in0=ps_ap, scalar=a_sc[:, k:k + 1],
                        in1=at, op0=MULT, op1=ADD,
                    )
                # bias madds folded into the k=1/k=2 combines (2 each):
                # off both the fill window and the store tail, and
                # spread so DVE stays under the 3.46 us pass budget
                if k in (1, 2):
                    for kk in ((0, 1) if k == 1 else (2, 3)):
                        nc.vector.scalar_tensor_tensor(
                            out=at, in0=bias_rep[:, kk, c0:c1],
                            scalar=a_sc[:, kk:kk + 1], in1=at,
                            op0=MULT, op1=ADD,
                        )

            # k=0 passes 0-2 skewed over the W0 granule stream: each
            # arriving 2-ii granule feeds 6 matmuls (3 tiles x 2 ii),
            # matching PE pace (~1.3 us) to granule delivery (~1.2 us)
            # so the PE does 3 passes of real work during the fill
            # instead of stalling on W0.
            # passes are staggered (granule lags 0/1/3) so the later
            # passes always consume granules that landed >=1.2 us ago,
            # absorbing DMA jitter (x2 also arrives ~2 us after x1)
            _NSKEW = 3
            _LAGS = (0, 1, 3)
            _NG0 = _NIT // G0
            ps_sk = [psump.tile([128, _OL], f32, tag="ps", name=f"ps0_{p}")
                     for p in range(_NSKEW)]
            for hs in range(_NG0 + _LAGS[-1]):
                for p in range(_NSKEW):
                    h = hs - _LAGS[p]
                    if not 0 <= h < _NG0:
                        continue
                    for ii in range(G0 * h, G0 * (h + 1)):
                        nc.tensor.matmul(
                            ps_sk[p], lhsT=xts[p][:, ii, :],
                            rhs=w_slice(0, ii),
                            start=(ii == 0), stop=(ii == _NIT - 1),
                        )
                if hs < 4:
                    # small dummy groups soak up granule-arrival jitter
                    # so the PE stays 100% busy until the HAM window
                    # fills; ~0.2 us each when data is on time
                    for j in range(4):
                        nc.tensor.matmul(
                            ps_warm[:, 0:128], lhsT=warm[:, 0:128],
                            rhs=warm[:, 0:128],
                            start=(j == 0), stop=(j == 3),
                        )
            for p in range(_NSKEW):
                acc[p] = accp.tile([128, _OL], f32, tag="acc",
                                   name=f"acc{p}")
                combine(0, p, ps_sk[p], attn_sb[:, p, :], 0, _OL, 0)

            # rest of the k=0 sweep (granule-paced -> dense)
            for t in range(_NSKEW, _NBT):
                xt = xts[t]
                a_sc = attn_sb[:, t, :]
                ps = psump.tile([128, _OL], f32, tag="ps", name=f"ps0_{t}")
                for ii in range(_NIT):
                    nc.tensor.matmul(
                        ps, lhsT=xt[:, ii, :], rhs=w_slice(0, ii),
                        start=(ii == 0), stop=(ii == _NIT - 1),
                    )
                if t < _NBT - 1:
                    acc[t] = accp.tile([128, _OL], f32, tag="acc",
                                       name=f"acc{t}")
                    combine(0, t, ps, a_sc, 0, _OL, 0)
                else:
                    acc15[0] = acc15p.tile([128, _SPLIT], f32, tag="accA",
                                           name="accA")
                    acc15[1] = acc15p.tile([128, _OL - _SPLIT], f32,
                                           tag="accB", name="accB")
                    combine(0, t, ps[:, 0:_SPLIT], a_sc, 0, _SPLIT, 0)
                    combine(0, t, ps[:, _SPLIT:_OL], a_sc, _SPLIT, _OL, 1)

            # fused k=1..3 sweep for t<15: one LDWEIGHTS (x slice) feeds
            # three matmuls into three open PSUM banks, cutting the NX
            # dispatch overhead (~5.9 ns/matmul, LDW+MM) by ~a third
            for t in range(_NBT - 1):
                xt = xts[t]
                a_sc = attn_sb[:, t, :]
                ps3 = [psump.tile([128, _OL], f32, tag="ps",
                                  name=f"ps{k}_{t}") for k in (1, 2, 3)]
                for ii in range(_NIT):
                    for j, k in enumerate((1, 2, 3)):
                        nc.tensor.matmul(
                            ps3[j], lhsT=xt[:, ii, :], rhs=w_slice(k, ii),
                            start=(ii == 0), stop=(ii == _NIT - 1),
                        )
                for j, k in enumerate((1, 2, 3)):
                    combine(k, t, ps3[j], a_sc, 0, _OL, 0)
                nc.sync.dma_start(
                    out=out[t * 128:(t + 1) * 128, :], in_=acc[t],
                )

            # t=15 unfused so its k=1/k=2 combines (with bias madds)
            # drain during the k=2/k=3 matmuls, keeping the tail at one
            # small combine + 64 KiB store
            xt = xts[_NBT - 1]
            a_sc = attn_sb[:, _NBT - 1, :]
            for k in (1, 2):
                ps = psump.tile([128, _OL], f32, tag="ps",
                                name=f"ps{k}_15")
                for ii in range(_NIT):
                    nc.tensor.matmul(
                        ps, lhsT=xt[:, ii, :], rhs=w_slice(k, ii),
                        start=(ii == 0), stop=(ii == _NIT - 1),
                    )
                combine(k, _NBT - 1, ps[:, 0:_SPLIT], a_sc, 0, _SPLIT, 0)
                combine(k, _NBT - 1, ps[:, _SPLIT:_OL], a_sc, _SPLIT,
                        _OL, 1)
            for which, (c0, c1) in enumerate(
                    [(0, _SPLIT), (_SPLIT, _OL)]):
                ph = psumhp.tile([128, c1 - c0], f32, tag=f"psh{which}",
                                 name=f"psh{which}")
                for ii in range(_NIT):
                    nc.tensor.matmul(
                        ph, lhsT=xt[:, ii, :],
                        rhs=w_slice(3, ii, c0, c1),
                        start=(ii == 0), stop=(ii == _NIT - 1),
                    )
                combine(3, _NBT - 1, ph, a_sc, c0, c1, which)
                q = nc.sync if which == 0 else nc.scalar
                q.dma_start(
                    out=out[(_NBT - 1) * 128:_NBT * 128, c0:c1],
                    in_=acc15[which],
                )

    nc.compile()
    return nc


def _get_program():
    if "nc" not in _CACHE:
        _CACHE["nc"] = _build_program()
    return _CACHE["nc"]


def _ensure_axon_hooks_importable():
    """bass_utils' trace branch imports antenv.axon_hooks, which the
    trimmed agent image may lack; stub it (hook=None) so a stray
    BASS_TRACE=1 degrades to an untraced run instead of crashing."""
    import sys
    import types

    try:
        import antenv.axon_hooks  # noqa: F401
        return
    except ImportError:
        pass
    mod = types.ModuleType("antenv.axon_hooks")
    mod._hook = None
    mod.get_axon_ntff_profile_hook = lambda: mod._hook

    def _set(h):
        mod._hook = h

    mod.set_axon_ntff_profile_hook = _set
    sys.modules["antenv.axon_hooks"] = mod
    try:
        import antenv
        antenv.axon_hooks = mod
    except ImportError:
        pass


def kernel(**inputs):
    global LAST_RESULTS
    from concourse.bass_utils import run_bass_kernel_spmd

    _ensure_axon_hooks_importable()

    x = np.ascontiguousarray(inputs["x"], dtype=np.float32)
    attn = np.ascontiguousarray(inputs["softmax_attention"], dtype=np.float32)
    w = np.ascontiguousarray(inputs["weight"], dtype=np.float32)
    b = np.ascontiguousarray(inputs["bias"], dtype=np.float32)

    nc = _get_program()
    in_maps = []
    for c in range(8):
        gb, go = divmod(c, _GRID_O)
        x_sl = x[gb * _BL:(gb + 1) * _BL]
        w_sl = w[:, go * _OL:(go + 1) * _OL, :]
        # tile-contiguous device layouts (see _build_program):
        # xT[t, i_in, ii, b_in] = x[t*128 + b_in, ii*128 + i_in]
        # wT[k, i_in, ii, o]    = W[k, o, ii*128 + i_in]
        import ml_dtypes
        xT = np.ascontiguousarray(
            x_sl.T.reshape(_NIT, 128, _NBT, 128).transpose(2, 1, 0, 3)
        ).astype(ml_dtypes.bfloat16)
        wTa = np.ascontiguousarray(
            w_sl.transpose(0, 2, 1)
            .reshape(_K, _NIT, 128, _OL).transpose(0, 2, 1, 3)
        ).astype(ml_dtypes.bfloat16)
        # attnT[p, t, k] = attn[gb*BL + t*128 + p, k]
        attnT = np.ascontiguousarray(
            attn[gb * _BL:(gb + 1) * _BL]
            .reshape(_NBT, 128, _K).transpose(1, 0, 2)
        )
        in_maps.append({
            "xT": xT,
            "attn": attnT,
            "wT": wTa,
            "bias": np.ascontiguousarray(np.broadcast_to(
                b[None, :, go * _OL:(go + 1) * _OL], (128, _K, _OL))),
        })

    res = run_bass_kernel_spmd(nc, in_maps, list(range(8)))
    LAST_RESULTS = res

    full = np.empty((_B, _OUT), dtype=np.float32)
    for c in range(8):
        gb, go = divmod(c, _GRID_O)
        full[gb * _BL:(gb + 1) * _BL, go * _OL:(go + 1) * _OL] = \
            res.results[c]["out"]
    return full
